# revision 7
# baseline (speedup 1.0000x reference)
"""Trainium2 Bass kernel for NeuralCausalModel (per-variable 3-layer MLP).

Math (reference):
    wx = x @ A.T                                   [B, V]
    comb_i = concat([x, wx[:, i]], -1)             [B, V+1]
    h1_i = relu(comb_i @ W1[i].T + b1[i])          [B, D]
    h2_i = relu(h1_i @ W2[i].T + b2[i])            [B, D]
    out[:, i] = h2_i @ W3[i] + b3[i]               [B]

Host-side fold: the concat column contributes wx[b,i]*W1[i][d,V] with
wx[b,i] = sum_k x[b,k] A[i,k], so
    W1eff[i][d,k] = W1[i][d,k] + W1[i][d,V] * A[i,k]
    -> h1_i = relu(x @ W1eff[i].T + b1[i])
which removes the ragged K=257 contraction and the adjacency matmul.

Sharding: variable axis V=256 split across 8 cores (32 vars/core),
x replicated, out gathered on host. No collectives.

Device layout: activations transposed [feature, batch] so biases are
per-partition scalars for ACT/DVE. Per variable:
    M1: h1T[d,b]  = relu(W1effT[i].T-chain @ xT)     4 Mtiles x 2 Bchunks x 2 Ktiles
    M2: h2T[e,b]  = relu(W2T[i]-chain @ h1T)         4 x 2 x 4
    M3: outT[1,b] = W3[i].T @ h2T + b3[i]            2 Bchunks x 4 Ktiles (M=1)
"""

import contextlib

import numpy as np

V, D, B = 256, 512, 1024
NCORES = 8
VL = V // NCORES  # 32 variables per core

# Matmul dtype: "f32r" (fp32 storage, reduced-precision single-pass matmul,
# 1 cycle/row at N>=512) or "bf16" or "f32".
import os as _os

MM_DTYPE = _os.environ.get("KERNEL_MM_DTYPE", "f32r")

_CACHE = {}


def _np_mm_dtype():
    if MM_DTYPE == "bf16":
        import ml_dtypes

        return ml_dtypes.bfloat16
    return np.float32


def _build(reps=1):
    key = (MM_DTYPE, reps)
    if key in _CACHE:
        return _CACHE[key]

    import sys

    if "/opt/trn_rl_repo" not in sys.path:
        sys.path.insert(0, "/opt/trn_rl_repo")

    import concourse.mybir as mybir
    import concourse.tile as tile
    from concourse import bacc

    f32 = mybir.dt.float32
    mdt = {
        "f32r": mybir.dt.float32r,
        "bf16": mybir.dt.bfloat16,
        "f32": mybir.dt.float32,
    }[MM_DTYPE]

    nc = bacc.Bacc("TRN2", target_bir_lowering=False, debug=False)

    xT = nc.declare_dram_parameter("xT", [V, B], mdt, isOutput=False)
    w1 = nc.declare_dram_parameter("w1t", [VL, V, D], mdt, isOutput=False)
    w2 = nc.declare_dram_parameter("w2t", [VL, D, D], mdt, isOutput=False)
    w3 = nc.declare_dram_parameter("w3t", [128, 128], mdt, isOutput=False)
    b1 = nc.declare_dram_parameter("b1t", [128, 128], f32, isOutput=False)
    b2 = nc.declare_dram_parameter("b2t", [128, 128], f32, isOutput=False)
    b3 = nc.declare_dram_parameter("b3t", [1, VL], f32, isOutput=False)
    out = nc.declare_dram_parameter("out", [VL, B], f32, isOutput=True)

    Relu = mybir.ActivationFunctionType.Relu
    Ident = mybir.ActivationFunctionType.Identity
    add = mybir.AluOpType.add
    amax = mybir.AluOpType.max

    with tile.TileContext(nc) as tc:
        with (
            tc.tile_pool(name="const", bufs=1) as const_pool,
            tc.tile_pool(name="w1p", bufs=6) as w1_pool,
            tc.tile_pool(name="w2p", bufs=12) as w2_pool,
            tc.tile_pool(name="h1p", bufs=8) as h1_pool,
            tc.tile_pool(name="h2p", bufs=8) as h2_pool,
            tc.tile_pool(name="psp", bufs=5, space="PSUM") as ps_pool,
            tc.tile_pool(name="ps3p", bufs=2, space="PSUM") as ps3_pool,
        ):
            xt0 = const_pool.tile([128, B], mdt, tag="xt0")
            xt1 = const_pool.tile([128, B], mdt, tag="xt1")
            nc.sync.dma_start(xt0[:], xT[0:128, :])
            nc.sync.dma_start(xt1[:], xT[128:256, :])
            b1sb = const_pool.tile([128, 128], f32, tag="b1sb")
            nc.sync.dma_start(b1sb[:], b1[:])
            b2sb = const_pool.tile([128, 128], f32, tag="b2sb")
            nc.sync.dma_start(b2sb[:], b2[:])
            w3sb = const_pool.tile([128, 128], mdt, tag="w3sb")
            nc.sync.dma_start(w3sb[:], w3[:])
            b3sb = const_pool.tile([1, VL], f32, tag="b3sb")
            nc.sync.dma_start(b3sb[:], b3[:])

            rep_ctx = tc.For_i(0, reps, 1) if reps > 1 else contextlib.nullcontext()
            with rep_ctx:
                for v in range(VL):
                    w1t = [
                        w1_pool.tile([128, D], mdt, tag="w1t", name=f"w1t_{k}")
                        for k in range(2)
                    ]
                    for kk in range(2):
                        nc.sync.dma_start(
                            w1t[kk][:], w1[v, kk * 128 : (kk + 1) * 128, :]
                        )
                    w2t = [
                        w2_pool.tile([128, D], mdt, tag="w2t", name=f"w2t_{k}")
                        for k in range(4)
                    ]
                    for dd in range(4):
                        nc.sync.dma_start(
                            w2t[dd][:], w2[v, dd * 128 : (dd + 1) * 128, :]
                        )

                    # ---- layer 1: h1T[d, b] = relu(W1eff[i] @ x.T + b1) ----
                    h1t = [
                        h1_pool.tile([128, B], mdt, tag="h1t", name=f"h1t_{k}")
                        for k in range(4)
                    ]
                    for bb in range(2):
                        bs = slice(bb * 512, (bb + 1) * 512)
                        for dd in range(4):
                            ms = slice(dd * 128, (dd + 1) * 128)
                            ps = ps_pool.tile([128, 512], f32, tag="ps", name="ps")
                            nc.tensor.matmul(
                                ps[:], w1t[0][:, ms], xt0[:, bs], start=True, stop=False
                            )
                            nc.tensor.matmul(
                                ps[:], w1t[1][:, ms], xt1[:, bs], start=False, stop=True
                            )
                            nc.scalar.activation(
                                h1t[dd][:, bs],
                                ps[:],
                                Relu,
                                bias=b1sb[:, v * 4 + dd : v * 4 + dd + 1],
                            )

                    # ---- layer 2: h2T[e, b] = relu(W2[i] @ h1 + b2) ----
                    h2t = [
                        h2_pool.tile([128, B], mdt, tag="h2t", name=f"h2t_{k}")
                        for k in range(4)
                    ]
                    for bb in range(2):
                        bs = slice(bb * 512, (bb + 1) * 512)
                        for ee in range(4):
                            ms = slice(ee * 128, (ee + 1) * 128)
                            ps = ps_pool.tile([128, 512], f32, tag="ps", name="ps")
                            for dd in range(4):
                                nc.tensor.matmul(
                                    ps[:],
                                    w2t[dd][:, ms],
                                    h1t[dd][:, bs],
                                    start=(dd == 0),
                                    stop=(dd == 3),
                                )
                            # bias + relu on DVE to offload the ACT engine
                            nc.vector.tensor_scalar(
                                h2t[ee][:, bs],
                                ps[:],
                                b2sb[:, v * 4 + ee : v * 4 + ee + 1],
                                0.0,
                                op0=add,
                                op1=amax,
                            )

                    # ---- layer 3: out[1, b] = W3[i] . h2 + b3 ----
                    m3sb = h1_pool.tile([1, B], f32, tag="m3sb", name="m3sb", bufs=4)
                    for bb in range(2):
                        bs = slice(bb * 512, (bb + 1) * 512)
                        ps3 = ps3_pool.tile([1, 512], f32, tag="ps3", name="ps3")
                        for ee in range(4):
                            nc.tensor.matmul(
                                ps3[:],
                                w3sb[:, v * 4 + ee : v * 4 + ee + 1],
                                h2t[ee][:, bs],
                                start=(ee == 0),
                                stop=(ee == 3),
                            )
                        nc.scalar.activation(
                            m3sb[0:1, bs],
                            ps3[:],
                            Ident,
                            bias=b3sb[0:1, v : v + 1],
                        )
                    nc.sync.dma_start(out[v : v + 1, :], m3sb[:])

    nc.compile()
    _CACHE[key] = nc
    return nc


def _prep_inputs(x, adjacency, W1, b1, W2, b2, W3, b3):
    """Host-side preprocessing + per-core sharding."""
    mmnp = _np_mm_dtype()
    x = np.asarray(x, np.float32)
    A = np.asarray(adjacency, np.float32)
    W1 = np.asarray(W1, np.float32)
    W2 = np.asarray(W2, np.float32)
    W3 = np.asarray(W3, np.float32)
    b1 = np.asarray(b1, np.float32)
    b2 = np.asarray(b2, np.float32)
    b3 = np.asarray(b3, np.float32)

    # Fold the concat column into W1.
    W1eff = W1[:, :, :V] + W1[:, :, V : V + 1] * A[:, None, :]  # [V, D, V]
    W1effT = np.ascontiguousarray(W1eff.transpose(0, 2, 1)).astype(mmnp)  # [i, k, d]
    W2T = np.ascontiguousarray(W2.transpose(0, 2, 1)).astype(mmnp)  # [i, d, e]
    xT = np.ascontiguousarray(x.T).astype(mmnp)  # [V, B]

    in_maps = []
    for c in range(NCORES):
        s = slice(c * VL, (c + 1) * VL)
        # bias/W3 tiles: [128, VL*4] with column i*4+j = (var i, feature tile j)
        b1t = np.ascontiguousarray(
            b1[s].reshape(VL, 4, 128).transpose(2, 0, 1).reshape(128, VL * 4)
        )
        b2t = np.ascontiguousarray(
            b2[s].reshape(VL, 4, 128).transpose(2, 0, 1).reshape(128, VL * 4)
        )
        w3t = np.ascontiguousarray(
            W3[s].reshape(VL, 4, 128).transpose(2, 0, 1).reshape(128, VL * 4)
        ).astype(mmnp)
        in_maps.append(
            {
                "xT": xT,
                "w1t": np.ascontiguousarray(W1effT[s]),
                "w2t": np.ascontiguousarray(W2T[s]),
                "w3t": w3t,
                "b1t": b1t,
                "b2t": b2t,
                "b3t": np.ascontiguousarray(b3[s].reshape(1, VL)),
            }
        )
    return in_maps


def kernel(x, adjacency, W1, b1, W2, b2, W3, b3):
    import sys

    if "/opt/trn_rl_repo" not in sys.path:
        sys.path.insert(0, "/opt/trn_rl_repo")
    from concourse.bass_utils import run_bass_kernel_spmd

    nc = _build()
    in_maps = _prep_inputs(x, adjacency, W1, b1, W2, b2, W3, b3)
    res = run_bass_kernel_spmd(nc, in_maps, core_ids=list(range(NCORES)))
    outT = np.concatenate([res.results[c]["out"] for c in range(NCORES)], axis=0)
    return np.ascontiguousarray(outT.T.astype(np.float32))
